# revision 2
# baseline (speedup 1.0000x reference)
"""Luong general attention (enc_len=2048, dec_len=512, B=32, H=256) on 8 trn2 cores.

Batch-sharded: core i handles batches 4i..4i+3. Per batch:
  projT = (dec_b @ W).T via PE transposes + fp32 matmul
  score[d,s] = projT.T @ encT (fp32) + rank-1 mask add (f32r) accumulated in PSUM
  softmax: DVE reduce_max -> ACT exp(bias=-max, accum_out=sum) -> DVE reciprocal
  weights_out = probs * (1/sum)  (DVE tensor_scalar)
  attn = probsT.T @ enc (float32r; probs/enc rounding ~2^-12 is far below tolerance)
"""
import numpy as np

ENC_LEN, DEC_LEN, B, H = 2048, 32, 512, 256  # note: B=32 total; per-core 4
BPC = 4  # batches per core
NCORES = 8

_CACHE = {}


def _build():
    import concourse.bacc as bacc
    import concourse.mybir as mybir
    import concourse.tile as tile

    f32 = mybir.dt.float32
    f32r = mybir.dt.float32r
    S, D, HD = 2048, 512, 256
    NS, ND = S // 128, D // 128  # 16 s-tiles, 4 d-tiles

    nc = bacc.Bacc("TRN2", target_bir_lowering=False, debug=False, num_devices=NCORES)
    enc_f = nc.dram_tensor("enc_f", [S, BPC, HD], f32, kind="ExternalInput").ap()
    enc_r = nc.dram_tensor("enc_r", [S, BPC, HD], f32r, kind="ExternalInput").ap()
    dec_d = nc.dram_tensor("dec", [D, BPC, HD], f32, kind="ExternalInput").ap()
    w_d = nc.dram_tensor("w", [HD, HD], f32, kind="ExternalInput").ap()
    id_d = nc.dram_tensor("ident", [128, 128], f32, kind="ExternalInput").ap()
    ones_d = nc.dram_tensor("ones", [1, 128], f32r, kind="ExternalInput").ap()
    mask_d = nc.dram_tensor("mask_add", [BPC, S], f32r, kind="ExternalInput").ap()
    attn_d = nc.dram_tensor("attn", [D, BPC, HD], f32, kind="ExternalOutput").ap()
    wout_d = nc.dram_tensor("wout", [BPC * D, S], f32, kind="ExternalOutput").ap()

    EXP = mybir.ActivationFunctionType.Exp
    X = mybir.AxisListType.X

    with tile.TileContext(nc) as tc:
        with (
            tc.tile_pool(name="cst", bufs=1) as cst,
            tc.tile_pool(name="io", bufs=2) as io,
            tc.tile_pool(name="mid", bufs=2) as mid,
            tc.tile_pool(name="soft", bufs=2) as soft,
            tc.tile_pool(name="score", bufs=1, space="PSUM") as score_ps,
            tc.tile_pool(name="tp", bufs=2, space="PSUM") as tp_ps,
            tc.tile_pool(name="acc", bufs=2, space="PSUM") as acc_ps,
        ):
            ident = cst.tile([128, 128], f32, tag="id")
            nc.sync.dma_start(out=ident[:], in_=id_d)
            ones = cst.tile([1, 128], f32r, tag="ones")
            nc.sync.dma_start(out=ones[:], in_=ones_d)
            w_sb = cst.tile([128, 2, HD], f32, tag="w")
            nc.sync.dma_start(out=w_sb[:], in_=w_d.rearrange("(c k) h -> k c h", c=2))

            for b in range(BPC):
                enc_nat = io.tile([128, NS, HD], f32, tag="enc")
                enc_natr = io.tile([128, NS, HD], f32r, tag="encr")
                # split into 4 DMAs for queue parallelism
                for q in range(4):
                    sl = enc_f[:, b, :].rearrange("(n p) h -> p n h", p=128)
                    nc.sync.dma_start(out=enc_nat[:, 4 * q:4 * q + 4, :],
                                      in_=sl[:, 4 * q:4 * q + 4, :])
                    slr = enc_r[:, b, :].rearrange("(n p) h -> p n h", p=128)
                    nc.sync.dma_start(out=enc_natr[:, 4 * q:4 * q + 4, :],
                                      in_=slr[:, 4 * q:4 * q + 4, :])
                dec_nat = io.tile([128, ND, HD], f32, tag="dec")
                nc.sync.dma_start(out=dec_nat[:],
                                  in_=dec_d[:, b, :].rearrange("(t p) h -> p t h", p=128))
                mrow = io.tile([1, S], f32r, tag="mrow")
                nc.sync.dma_start(out=mrow[:], in_=mask_d[b:b + 1, :])

                # decT[hd, d] via PE transposes
                decT = mid.tile([128, 2, D], f32, tag="decT")
                for j in range(2):
                    tp = tp_ps.tile([128, 512], f32, tag="tp")
                    for t in range(ND):
                        nc.tensor.transpose(tp[:, 128 * t:128 * t + 128],
                                            dec_nat[:, t, 128 * j:128 * j + 128], ident[:])
                    nc.vector.tensor_copy(decT[:, j, :], tp[:])

                # projT[he, d] = W.T-chunks @ decT  (fp32)
                projT = mid.tile([128, 2, D], f32, tag="projT")
                for j in range(2):
                    pj = tp_ps.tile([128, 512], f32, tag="tp")
                    for k in range(2):
                        nc.tensor.matmul(pj[:], w_sb[:, k, 128 * j:128 * j + 128],
                                         decT[:, k, :], start=(k == 0), stop=(k == 1))
                    nc.vector.tensor_copy(projT[:, j, :], pj[:])

                # encT[he, s] via PE transposes
                encT = mid.tile([128, 2, S], f32, tag="encT")
                for j in range(2):
                    for g in range(4):
                        tp = tp_ps.tile([128, 512], f32, tag="tp")
                        for u in range(4):
                            n = 4 * g + u
                            nc.tensor.transpose(tp[:, 128 * u:128 * u + 128],
                                                enc_nat[:, n, 128 * j:128 * j + 128], ident[:])
                        nc.vector.tensor_copy(encT[:, j, 512 * g:512 * g + 512], tp[:])

                for dt in range(ND):
                    sc = score_ps.tile([128, S], f32, tag="sc")
                    for s4 in range(4):
                        seg = slice(512 * s4, 512 * s4 + 512)
                        for k in range(2):
                            nc.tensor.matmul(sc[:, seg],
                                             projT[:, k, 128 * dt:128 * dt + 128],
                                             encT[:, k, seg], start=(k == 0), stop=False)
                        nc.tensor.matmul(sc[:, seg], ones[:], mrow[:, seg],
                                         start=False, stop=True)
                    rmax = soft.tile([128, 1], f32, tag="rmax")
                    nc.vector.reduce_max(rmax[:], sc[:], axis=X)
                    negmax = soft.tile([128, 1], f32, tag="negmax")
                    nc.vector.tensor_scalar_mul(negmax[:], rmax[:], -1.0)
                    probs = soft.tile([128, S], f32, tag="probs")
                    rsum = soft.tile([128, 1], f32, tag="rsum")
                    nc.scalar.activation(probs[:], sc[:], EXP,
                                         bias=negmax[:], scale=1.0, accum_out=rsum[:])
                    rrec = soft.tile([128, 1], f32, tag="rrec")
                    nc.vector.reciprocal(rrec[:], rsum[:])
                    wn = soft.tile([128, S], f32, tag="wn")
                    nc.vector.tensor_scalar_mul(wn[:], probs[:], rrec[:])
                    row0 = b * D + dt * 128
                    nc.sync.dma_start(out=wout_d[row0:row0 + 128, :], in_=wn[:])

                    # probsT (f32r) then attn += probsT.T @ enc_r
                    probsT = soft.tile([128, NS, 128], f32r, tag="probsT")
                    for g in range(4):
                        tp = tp_ps.tile([128, 512], f32, tag="tp")
                        for u in range(4):
                            n = 4 * g + u
                            nc.tensor.transpose(tp[:, 128 * u:128 * u + 128],
                                                probs[:, 128 * n:128 * n + 128], ident[:])
                        nc.scalar.copy(probsT[:, 4 * g:4 * g + 4, :].rearrange("p a c -> p (a c)"),
                                       tp[:])
                    ap = acc_ps.tile([128, HD], f32, tag="attn")
                    for n in range(NS):
                        nc.tensor.matmul(ap[:], probsT[:, n, :], enc_natr[:, n, :],
                                         start=(n == 0), stop=(n == NS - 1))
                    attn_sb = soft.tile([128, HD], f32, tag="attn_sb")
                    nc.vector.tensor_scalar_mul(attn_sb[:], ap[:], rrec[:])
                    nc.sync.dma_start(out=attn_d[128 * dt:128 * dt + 128, b, :], in_=attn_sb[:])
    nc.compile()
    return nc


def _get_nc():
    if "nc" not in _CACHE:
        _CACHE["nc"] = _build()
    return _CACHE["nc"]


def _in_maps(encoder_outputs, decoder_outputs, general_weights, enc_mask):
    enc = np.ascontiguousarray(encoder_outputs, dtype=np.float32)
    dec = np.ascontiguousarray(decoder_outputs, dtype=np.float32)
    w = np.ascontiguousarray(general_weights, dtype=np.float32)
    mask_add = (enc_mask.T.astype(np.float32) - 1.0) * 1.0e12  # (32, 2048)
    ident = np.eye(128, dtype=np.float32)
    ones = np.ones((1, 128), dtype=np.float32)
    maps = []
    for i in range(NCORES):
        bs = slice(BPC * i, BPC * i + BPC)
        e = np.ascontiguousarray(enc[:, bs, :])
        maps.append({
            "enc_f": e, "enc_r": e,
            "dec": np.ascontiguousarray(dec[:, bs, :]),
            "w": w, "ident": ident, "ones": ones,
            "mask_add": np.ascontiguousarray(mask_add[bs, :]),
        })
    return maps


def run(trace=False, **inputs):
    from concourse.bass_utils import run_bass_kernel_spmd
    nc = _get_nc()
    maps = _in_maps(**inputs)
    res = run_bass_kernel_spmd(nc, maps, list(range(NCORES)), trace=trace)
    attn = np.concatenate([res.results[i]["attn"] for i in range(NCORES)], axis=1)
    weights = np.concatenate([res.results[i]["wout"] for i in range(NCORES)], axis=0)
    return (attn, weights), res


def kernel(**inputs):
    out, _ = run(trace=False, **inputs)
    return out
